# revision 18
# baseline (speedup 1.0000x reference)
"""Self-attention kernel for Trainium2 (8 NeuronCores, SPMD).

Problem: X[8192,512], Wq,Wk[512,512]:
    Q = X@Wq ; K = X@Wk ; S = softmax(Q K^T / sqrt(512)) ; out = S @ X

Sharding: rows of Q (query blocks of 1024) across 8 cores; K/V (=X) replicated.

Math restructure: S^T = X (Wk Wq^T) Xm^T / sqrt(D), so per core:
  Phase A:  A^T = Wq Wk^T / sqrt(D)            [512,512]  (16 matmuls)
  Phase R:  R   = A Xm^T                       [512,1024] (resident, f32r)
  Per i-half h (512 query columns):
    B1: stream X^T blocks; S^T tile [j=128, i=512] = xt_chunk.T @ R
        (4 accumulating f32r matmuls) -> ACT copy PSUM->SBUF (S region),
        DVE running elementwise max -> mx[128,512]
    FIN: partition-reduce mx via PE transpose + DVE reduce_max, broadcast
        back to [128,512] via ones outer-product matmul -> b
    B3: per j-tile: d = max(S^T,-80) - b (one fused DVE op), exp (ACT, f16
        out); PE: 4 accumulating matmuls o[i-chunk,512] += P~[:,chunk].T
        @ X[j-tile]; gpsimd running-accumulates P~ into acc[128,512]
    B4: partition-reduce acc (PE transpose + DVE reduce_sum) -> row sums,
        reciprocal on [128,4] (per-partition, fast), drain o PSUM->SBUF
        with per-partition scale split across ACT and DVE, DMA out.

Block order is host-permuted per core (own 2 blocks first/last) so the
core's own X^T blocks are loaded once, serve as R's rhs AND as two of the
16 B1 blocks in both halves; reductions over j are order-invariant.
The host supplies X^T f32 and X f16 in blocked layouts (staging only; all
FLOPs happen on device). fp32r matmuls keep ~13 mantissa bits => logit
noise ~0.1 => output rel err ~6e-3.
"""
import sys

sys.path.insert(0, "/opt/trn_rl_repo")

import numpy as np

import concourse.bass as bass
import concourse.mybir as mybir
import concourse.tile as tile
from concourse import bacc
from concourse.bass import ts
from concourse.bass_utils import run_bass_kernel_spmd
from concourse.masks import make_identity

F32 = mybir.dt.float32
F32R = mybir.dt.float32r
F16 = mybir.dt.float16
AF = mybir.ActivationFunctionType
ALU = mybir.AluOpType

N = 8192
D = 512
NCORES = 8
MY_N = N // NCORES          # 1024 query rows per core
NBLK = N // 512             # 16 j-blocks of 512 rows
NJT = N // 128              # 64 j-tiles
NIH = MY_N // 512           # 2 i-halves
CLAMP = -80.0

# j-block processing order (indices into the host-permuted block axis):
# permuted block 0 = my first block (resident, no DMA) -> warm start;
# permuted block 1 = my second block (resident) -> placed last so the
# stream DMA finishes early and the B3 x-stream can begin sooner.
ORDER = [0] + list(range(2, NBLK)) + [1]

_NC_CACHE = None


def _build_nc():
    nc = bacc.Bacc(None, target_bir_lowering=False)

    xts = nc.dram_tensor("xts", [128, NBLK, 4, 512], F32R, kind="ExternalInput")
    xsb = nc.dram_tensor("xsb", [128, NBLK, 4, 512], F16, kind="ExternalInput")
    wqt = nc.dram_tensor("wqt", [D, D], F32R, kind="ExternalInput")   # Wq^T
    wkt = nc.dram_tensor("wkt", [D, D], F32R, kind="ExternalInput")   # Wk^T
    o = nc.dram_tensor("o", [MY_N, D], F32, kind="ExternalOutput")

    with tile.TileContext(nc) as tc:
        with (
            tc.tile_pool(name="pool", bufs=1) as pool,          # persistent
            tc.tile_pool(name="wp", bufs=3) as wp,              # weights -> stream
            tc.tile_pool(name="myp", bufs=1) as myp,            # my 2 X^T blocks
            tc.tile_pool(name="rp", bufs=1) as rp,              # R
            tc.tile_pool(name="big", bufs=1) as big,            # S^T region
            tc.tile_pool(name="xs", bufs=2) as xsp,             # X f16 tiles (B3)
            tc.tile_pool(name="wd", bufs=2) as wd,              # d / output drain
            tc.tile_pool(name="wpp", bufs=2) as wpp,            # p
            tc.tile_pool(name="wtp", bufs=2) as wtp,            # p pair sums
            tc.tile_pool(name="ps_qk", bufs=3, space="PSUM") as ps_qk,
            tc.tile_pool(name="ps_o", bufs=1, space="PSUM") as ps_o,
            tc.tile_pool(name="ps_sum", bufs=1, space="PSUM") as ps_sum,
        ):
            # ---- constants ----
            ident = pool.tile([128, 128], F32)
            make_identity(nc, ident[:])
            ones_f32 = pool.tile([128, 2], F32)
            nc.vector.memset(ones_f32[:], 1.0)
            ones_col = pool.tile([128, 1], F16)    # lhsT for pair column sums
            nc.vector.tensor_copy(ones_col[:], ones_f32[:, 0:1])
            ones_row_f32 = pool.tile([1, 128], F32)
            nc.vector.memset(ones_row_f32[:], 1.0)
            ones_row = pool.tile([1, 128], F32R)   # lhsT for broadcast
            nc.vector.tensor_copy(ones_row[:], ones_row_f32[:])
            one_one = pool.tile([1, 1], F32)
            nc.vector.memset(one_one[:], 1.0)

            # ---- input DMAs (weights first: they gate the A matmuls; my
            # X^T blocks split so R's first half can start sooner) ----
            wqt_sb = wp.tile([128, 4, D], F32R, tag="w")
            nc.sync.dma_start(wqt_sb[:], wqt[:].rearrange("(c p) d -> p c d", p=128))
            wkt_sb = wp.tile([128, 4, D], F32R, tag="w")
            nc.sync.dma_start(wkt_sb[:], wkt[:].rearrange("(c p) d -> p c d", p=128))
            myblk = myp.tile([128, 2, 4, 512], F32R, tag="my")
            nc.sync.dma_start(myblk[:, 0], xts[:, 0, :, :])
            nc.sync.dma_start(myblk[:, 1], xts[:, 1, :, :])

            # ---- Phase A: A^T = (Wq Wk^T) / sqrt(D) ----
            scale = 1.0 / float(np.sqrt(D))
            at_sb = wp.tile([128, 4, D], F32R, tag="w")
            for cc in range(4):
                a_ps = ps_qk.tile([128, 512], F32, tag="qk")
                for b in range(4):
                    nc.tensor.matmul(
                        a_ps[:],
                        wqt_sb[:, b, ts(cc, 128)],
                        wkt_sb[:, b, :],
                        start=(b == 0),
                        stop=(b == 3),
                    )
                nc.scalar.activation(
                    at_sb[:, cc, :], a_ps[:], AF.Copy, bias=0.0, scale=scale
                )

            # ---- Phase R: R = A Xm^T  (so S^T tiles = xt_chunk.T @ R) ----
            r_sb = rp.tile([128, 4, MY_N], F32R, tag="r")

            def r_half(ih):
                for dch in range(4):
                    r_ps = ps_qk.tile([128, 512], F32, tag="qk")
                    for cc in range(4):
                        nc.tensor.matmul(
                            r_ps[:],
                            at_sb[:, cc, ts(dch, 128)],
                            myblk[:, ih, cc, :],
                            start=(cc == 0),
                            stop=(cc == 3),
                        )
                    nc.scalar.copy(r_sb[:, dch, ts(ih, 512)], r_ps[:])

            r_half(0)

            def b1_block(h, st, mx, pos, blk):
                if blk < 2:
                    xt_blk = myblk[:, blk, :, :]
                else:
                    xt_t = wp.tile([128, 4, 512], F32R, tag="w")
                    nc.sync.dma_start(xt_t[:], xts[:, blk, :, :])
                    xt_blk = xt_t[:]
                for t in range(4):
                    jt = pos * 4 + t
                    s_ps = ps_qk.tile([128, 512], F32, tag="qk")
                    for e in range(4):
                        nc.tensor.matmul(
                            s_ps[:],
                            xt_blk[:, e, ts(t, 128)],
                            r_sb[:, e, ts(h, 512)],
                            start=(e == 0),
                            stop=(e == 3),
                        )
                    nc.scalar.copy(st[:, jt, :], s_ps[:])
                    if jt == 0:
                        nc.vector.tensor_copy(mx[:], s_ps[:])
                    else:
                        nc.vector.tensor_tensor(mx[:], mx[:], s_ps[:], op=ALU.max)

            def finalize_max(mx):
                """mx[128,512] -> b_sb[128,512] broadcast of per-i max."""
                mcol = pool.tile([128, 4], F32, tag="mcol")
                for cc in range(4):
                    mt_ps = ps_qk.tile([128, 128], F32, tag="qk")
                    nc.tensor.transpose(mt_ps[:], mx[:, ts(cc, 128)], ident[:])
                    nc.vector.reduce_max(
                        mcol[:, cc : cc + 1], mt_ps[:], axis=mybir.AxisListType.X
                    )
                mrow_ps = ps_qk.tile([1, 512], F32, tag="qk")
                for cc in range(4):
                    nc.tensor.transpose(
                        mrow_ps[:, ts(cc, 128)], mcol[:, cc : cc + 1], ident[:]
                    )
                mrow = pool.tile([1, 512], F32R, tag="row")
                nc.scalar.copy(mrow[:], mrow_ps[:])
                b_ps = ps_qk.tile([128, 512], F32, tag="qk")
                nc.tensor.matmul(b_ps[:], ones_row[:], mrow[:], start=True, stop=True)
                # duplicated along the pair axis so one DVE op covers 2 tiles;
                # the two copies run on different engines in parallel
                b2_sb = pool.tile([128, 2, 512], F32, tag="bsb")
                nc.scalar.copy(b2_sb[:, 0, :], b_ps[:])
                nc.vector.tensor_copy(b2_sb[:, 1, :], b_ps[:])
                return b2_sb

            def b3_exp_and_accum(st, b2_sb, o_ps, sum_ps):
                # pair granularity: one DVE sub and one ACT exp per 2 j-tiles
                # amortizes per-instruction overheads and keeps both engines
                # comfortably ahead of the PE's 8 matmuls + 1 sum-matmul
                for pos, blk in enumerate(ORDER):
                    for half in range(2):
                        pr = pos * 2 + half
                        jt0 = pr * 2
                        x_blk = xsp.tile([128, 2, 512], F16, tag="x")
                        nc.sync.dma_start(
                            x_blk[:], xsb[:, blk, 2 * half : 2 * half + 2, :]
                        )
                        d2 = wd.tile([128, 2, 512], F16, tag="d")
                        nc.vector.tensor_tensor(
                            d2[:], st[:, jt0 : jt0 + 2, :], b2_sb[:],
                            op=ALU.subtract,
                        )
                        p2 = wpp.tile([128, 2, 512], F16, tag="p")
                        nc.scalar.activation(p2[:], d2[:], AF.Exp)
                        tmp = wtp.tile([128, 512], F16, tag="t")
                        nc.vector.tensor_tensor(
                            tmp[:], p2[:, 0, :], p2[:, 1, :], op=ALU.add
                        )
                        nc.tensor.matmul(
                            sum_ps[:],
                            ones_col[:],
                            tmp[:],
                            start=(pr == 0),
                            stop=(pr == NJT // 2 - 1),
                        )
                        for tix in range(2):
                            for cc in range(4):
                                nc.tensor.matmul(
                                    o_ps[:, cc, :],
                                    p2[:, tix, ts(cc, 128)],
                                    x_blk[:, tix, :],
                                    start=(jt0 + tix == 0),
                                    stop=(jt0 + tix == NJT - 1),
                                )

            def b4_drain(h, o_ps, sum_ps):
                srow = pool.tile([1, 512], F32, tag="row")
                nc.scalar.copy(srow[:], sum_ps[:])
                scol_ps = ps_qk.tile([128, 4], F32, tag="qk")
                for cc in range(4):
                    nc.tensor.transpose(
                        scol_ps[:, cc : cc + 1], srow[:, ts(cc, 128)], one_one[:]
                    )
                rcol = pool.tile([128, 4], F32, tag="rcol")
                nc.vector.reciprocal(rcol[:], scol_ps[:])
                for cc in range(4):
                    o_sb = wd.tile([128, 512], F32, tag="d")
                    if cc < 2:
                        nc.scalar.activation(
                            o_sb[:], o_ps[:, cc, :], AF.Copy,
                            bias=0.0, scale=rcol[:, cc : cc + 1],
                        )
                    else:
                        nc.vector.tensor_scalar_mul(
                            o_sb[:], o_ps[:, cc, :], rcol[:, cc : cc + 1]
                        )
                    nc.sync.dma_start(o[ts(h * 4 + cc, 128), :], o_sb[:])

            for h in range(NIH):
                st = big.tile([128, NJT, 512], F32, tag="big")
                mx = pool.tile([128, 512], F32, tag="mx")
                if h == 0:
                    # first block is resident (no stream slot), so R's second
                    # half can slot in behind it while its DMA completes
                    b1_block(h, st, mx, 0, ORDER[0])
                    r_half(1)
                    for pos in range(1, NBLK):
                        b1_block(h, st, mx, pos, ORDER[pos])
                else:
                    for pos in range(NBLK):
                        b1_block(h, st, mx, pos, ORDER[pos])
                b2_sb = finalize_max(mx)
                o_ps = ps_o.tile([128, 4, 512], F32, tag="o")
                sum_ps = ps_sum.tile([1, 512], F32, tag="sum")
                b3_exp_and_accum(st, b2_sb, o_ps, sum_ps)
                b4_drain(h, o_ps, sum_ps)

    nc.compile()
    return nc


def _get_nc():
    global _NC_CACHE
    if _NC_CACHE is None:
        _NC_CACHE = _build_nc()
    return _NC_CACHE


def kernel(rotation_params, entangle_params, inputs, _trace=False, _trace_kwargs=None):
    X = np.ascontiguousarray(inputs, dtype=np.float32)
    Wq = np.ascontiguousarray(rotation_params, dtype=np.float32)
    Wk = np.ascontiguousarray(entangle_params, dtype=np.float32)
    XT = np.ascontiguousarray(X.T)
    # blocked layouts: [p, blk, c, j] with 8KiB (f32) / 4KiB (f16) runs/partition
    XTB = XT.reshape(4, 128, NBLK, 512).transpose(1, 2, 0, 3)
    X16B = X.astype(np.float16).reshape(NBLK, 4, 128, 512).transpose(2, 0, 1, 3)
    wqt = np.ascontiguousarray(Wq.T)
    wkt = np.ascontiguousarray(Wk.T)

    in_maps = []
    for c in range(NCORES):
        # rotate the block axis so this core's own 2 blocks land at 0,1
        in_maps.append(
            {
                "xts": np.ascontiguousarray(np.roll(XTB, -2 * c, axis=1)),
                "xsb": np.ascontiguousarray(np.roll(X16B, -2 * c, axis=1)),
                "wqt": wqt,
                "wkt": wkt,
            }
        )

    nc = _get_nc()
    kw = {}
    if _trace:
        kw["trace"] = True
        kw.update(_trace_kwargs or {})
    br = run_bass_kernel_spmd(nc, in_maps, core_ids=list(range(NCORES)), **kw)
    out = np.concatenate([r["o"] for r in br.results], axis=0)
    if _trace:
        return out, br
    return out


# revision 19
# speedup vs baseline: 1.0450x; 1.0450x over previous
"""Self-attention kernel for Trainium2 (8 NeuronCores, SPMD).

Problem: X[8192,512], Wq,Wk[512,512]:
    Q = X@Wq ; K = X@Wk ; S = softmax(Q K^T / sqrt(512)) ; out = S @ X

Sharding: rows of Q (query blocks of 1024) across 8 cores; K/V (=X) replicated.

Math restructure: S^T = X (Wk Wq^T) Xm^T / sqrt(D), so per core:
  Phase A:  A^T = Wq Wk^T / sqrt(D)            [512,512]  (16 matmuls)
  Phase R:  R   = A Xm^T                       [512,1024] (resident, f32r)
  Per i-half h (512 query columns):
    B1: stream X^T blocks; S^T tile [j=128, i=512] = xt_chunk.T @ R
        (4 accumulating f32r matmuls) -> ACT copy PSUM->SBUF (S region),
        DVE running elementwise max -> mx[128,512]
    FIN: partition-reduce mx via PE transpose + DVE reduce_max, broadcast
        back to [128,512] via ones outer-product matmul -> b
    B3: per j-tile: d = max(S^T,-80) - b (one fused DVE op), exp (ACT, f16
        out); PE: 4 accumulating matmuls o[i-chunk,512] += P~[:,chunk].T
        @ X[j-tile]; gpsimd running-accumulates P~ into acc[128,512]
    B4: partition-reduce acc (PE transpose + DVE reduce_sum) -> row sums,
        reciprocal on [128,4] (per-partition, fast), drain o PSUM->SBUF
        with per-partition scale split across ACT and DVE, DMA out.

Block order is host-permuted per core (own 2 blocks first/last) so the
core's own X^T blocks are loaded once, serve as R's rhs AND as two of the
16 B1 blocks in both halves; reductions over j are order-invariant.
The host supplies X^T f32 and X f16 in blocked layouts (staging only; all
FLOPs happen on device). fp32r matmuls keep ~13 mantissa bits => logit
noise ~0.1 => output rel err ~6e-3.
"""
import sys

sys.path.insert(0, "/opt/trn_rl_repo")

import numpy as np

import concourse.bass as bass
import concourse.mybir as mybir
import concourse.tile as tile
from concourse import bacc
from concourse.bass import ts
from concourse.bass_utils import run_bass_kernel_spmd
from concourse.masks import make_identity

F32 = mybir.dt.float32
F32R = mybir.dt.float32r
F16 = mybir.dt.float16
AF = mybir.ActivationFunctionType
ALU = mybir.AluOpType

N = 8192
D = 512
NCORES = 8
MY_N = N // NCORES          # 1024 query rows per core
NBLK = N // 512             # 16 j-blocks of 512 rows
NJT = N // 128              # 64 j-tiles
NIH = MY_N // 512           # 2 i-halves
CLAMP = -80.0

# j-block processing order (indices into the host-permuted block axis):
# permuted block 0 = my first block (resident, no DMA) -> warm start;
# permuted block 1 = my second block (resident) -> placed last so the
# stream DMA finishes early and the B3 x-stream can begin sooner.
ORDER = [0] + list(range(2, NBLK)) + [1]

_NC_CACHE = None


def _build_nc():
    nc = bacc.Bacc(None, target_bir_lowering=False)

    xts = nc.dram_tensor("xts", [128, NBLK, 4, 512], F32R, kind="ExternalInput")
    xsb = nc.dram_tensor("xsb", [128, NBLK, 4, 512], F16, kind="ExternalInput")
    wqt = nc.dram_tensor("wqt", [D, D], F32R, kind="ExternalInput")   # Wq^T
    wkt = nc.dram_tensor("wkt", [D, D], F32R, kind="ExternalInput")   # Wk^T
    o = nc.dram_tensor("o", [MY_N, D], F32, kind="ExternalOutput")

    with tile.TileContext(nc) as tc:
        with (
            tc.tile_pool(name="pool", bufs=1) as pool,          # persistent
            tc.tile_pool(name="wp", bufs=3) as wp,              # weights -> stream
            tc.tile_pool(name="myp", bufs=1) as myp,            # my 2 X^T blocks
            tc.tile_pool(name="rp", bufs=1) as rp,              # R
            tc.tile_pool(name="big", bufs=1) as big,            # S^T region
            tc.tile_pool(name="xs", bufs=2) as xsp,             # X f16 tiles (B3)
            tc.tile_pool(name="wd", bufs=2) as wd,              # d / output drain
            tc.tile_pool(name="wpp", bufs=2) as wpp,            # p
            tc.tile_pool(name="wtp", bufs=2) as wtp,            # p pair sums
            tc.tile_pool(name="ps_qk", bufs=3, space="PSUM") as ps_qk,
            tc.tile_pool(name="ps_o", bufs=1, space="PSUM") as ps_o,
            tc.tile_pool(name="ps_sum", bufs=1, space="PSUM") as ps_sum,
        ):
            # ---- constants ----
            ident = pool.tile([128, 128], F32)
            make_identity(nc, ident[:])
            ones_f32 = pool.tile([128, 2], F32)
            nc.vector.memset(ones_f32[:], 1.0)
            ones_col = pool.tile([128, 1], F16)    # lhsT for pair column sums
            nc.vector.tensor_copy(ones_col[:], ones_f32[:, 0:1])
            ones_row_f32 = pool.tile([1, 128], F32)
            nc.vector.memset(ones_row_f32[:], 1.0)
            ones_row = pool.tile([1, 128], F32R)   # lhsT for broadcast
            nc.vector.tensor_copy(ones_row[:], ones_row_f32[:])
            one_one = pool.tile([1, 1], F32)
            nc.vector.memset(one_one[:], 1.0)

            # ---- input DMAs (weights first: they gate the A matmuls; my
            # X^T blocks split so R's first half can start sooner) ----
            wqt_sb = wp.tile([128, 4, D], F32R, tag="w")
            nc.sync.dma_start(wqt_sb[:], wqt[:].rearrange("(c p) d -> p c d", p=128))
            wkt_sb = wp.tile([128, 4, D], F32R, tag="w")
            nc.sync.dma_start(wkt_sb[:], wkt[:].rearrange("(c p) d -> p c d", p=128))
            myblk = myp.tile([128, 2, 4, 512], F32R, tag="my")
            nc.sync.dma_start(myblk[:, 0], xts[:, 0, :, :])
            nc.sync.dma_start(myblk[:, 1], xts[:, 1, :, :])

            # ---- Phase A: A^T = (Wq Wk^T) / sqrt(D) ----
            scale = 1.0 / float(np.sqrt(D))
            at_sb = wp.tile([128, 4, D], F32R, tag="w")
            for cc in range(4):
                a_ps = ps_qk.tile([128, 512], F32, tag="qk")
                for b in range(4):
                    nc.tensor.matmul(
                        a_ps[:],
                        wqt_sb[:, b, ts(cc, 128)],
                        wkt_sb[:, b, :],
                        start=(b == 0),
                        stop=(b == 3),
                    )
                nc.scalar.activation(
                    at_sb[:, cc, :], a_ps[:], AF.Copy, bias=0.0, scale=scale
                )

            # ---- Phase R: R = A Xm^T  (so S^T tiles = xt_chunk.T @ R) ----
            r_sb = rp.tile([128, 4, MY_N], F32R, tag="r")

            def r_half(ih):
                for dch in range(4):
                    r_ps = ps_qk.tile([128, 512], F32, tag="qk")
                    for cc in range(4):
                        nc.tensor.matmul(
                            r_ps[:],
                            at_sb[:, cc, ts(dch, 128)],
                            myblk[:, ih, cc, :],
                            start=(cc == 0),
                            stop=(cc == 3),
                        )
                    nc.scalar.copy(r_sb[:, dch, ts(ih, 512)], r_ps[:])

            r_half(0)

            def b1_block(h, st, mx, pos, blk):
                if blk < 2:
                    xt_blk = myblk[:, blk, :, :]
                else:
                    xt_t = wp.tile([128, 4, 512], F32R, tag="w")
                    nc.sync.dma_start(xt_t[:], xts[:, blk, :, :])
                    xt_blk = xt_t[:]
                for t in range(4):
                    jt = pos * 4 + t
                    s_ps = ps_qk.tile([128, 512], F32, tag="qk")
                    for e in range(4):
                        nc.tensor.matmul(
                            s_ps[:],
                            xt_blk[:, e, ts(t, 128)],
                            r_sb[:, e, ts(h, 512)],
                            start=(e == 0),
                            stop=(e == 3),
                        )
                    nc.scalar.copy(st[:, jt, :], s_ps[:])
                    if jt == 0:
                        nc.vector.tensor_copy(mx[:], s_ps[:])
                    else:
                        nc.vector.tensor_tensor(mx[:], mx[:], s_ps[:], op=ALU.max)

            def finalize_max(mx):
                """mx[128,512] -> b_sb[128,512] broadcast of per-i max."""
                mcol = pool.tile([128, 4], F32, tag="mcol")
                for cc in range(4):
                    mt_ps = ps_qk.tile([128, 128], F32, tag="qk")
                    nc.tensor.transpose(mt_ps[:], mx[:, ts(cc, 128)], ident[:])
                    nc.vector.reduce_max(
                        mcol[:, cc : cc + 1], mt_ps[:], axis=mybir.AxisListType.X
                    )
                mrow_ps = ps_qk.tile([1, 512], F32, tag="qk")
                for cc in range(4):
                    nc.tensor.transpose(
                        mrow_ps[:, ts(cc, 128)], mcol[:, cc : cc + 1], ident[:]
                    )
                mrow = pool.tile([1, 512], F32R, tag="row")
                nc.scalar.copy(mrow[:], mrow_ps[:])
                b_ps = ps_qk.tile([128, 512], F32, tag="qk")
                nc.tensor.matmul(b_ps[:], ones_row[:], mrow[:], start=True, stop=True)
                # duplicated along the pair axis so one DVE op covers 2 tiles;
                # the two copies run on different engines in parallel
                b2_sb = pool.tile([128, 2, 512], F32, tag="bsb")
                nc.scalar.copy(b2_sb[:, 0, :], b_ps[:])
                nc.vector.tensor_copy(b2_sb[:, 1, :], b_ps[:])
                return b2_sb

            def b3_exp_and_accum(st, b2_sb, o_ps, sum_ps):
                # pair granularity: one DVE sub and one ACT exp per 2 j-tiles
                # amortizes per-instruction overheads. The pair-add for the
                # row sums is deferred by one pair so the DVE never sits
                # behind an ACT exp round-trip in its own program order.
                NPR = NJT // 2

                def pair_add(pr, p2_prev):
                    tmp = wtp.tile([128, 512], F16, tag="t")
                    nc.vector.tensor_tensor(
                        tmp[:], p2_prev[:, 0, :], p2_prev[:, 1, :], op=ALU.add
                    )
                    nc.tensor.matmul(
                        sum_ps[:],
                        ones_col[:],
                        tmp[:],
                        start=(pr == 0),
                        stop=(pr == NPR - 1),
                    )

                p2_prev = None
                for pos, blk in enumerate(ORDER):
                    for half in range(2):
                        pr = pos * 2 + half
                        jt0 = pr * 2
                        x_blk = xsp.tile([128, 2, 512], F16, tag="x")
                        nc.sync.dma_start(
                            x_blk[:], xsb[:, blk, 2 * half : 2 * half + 2, :]
                        )
                        d2 = wd.tile([128, 2, 512], F16, tag="d")
                        nc.vector.tensor_tensor(
                            d2[:], st[:, jt0 : jt0 + 2, :], b2_sb[:],
                            op=ALU.subtract,
                        )
                        if pr > 0:
                            pair_add(pr - 1, p2_prev)
                        p2 = wpp.tile([128, 2, 512], F16, tag="p")
                        nc.scalar.activation(p2[:], d2[:], AF.Exp)
                        p2_prev = p2
                        for tix in range(2):
                            for cc in range(4):
                                nc.tensor.matmul(
                                    o_ps[:, cc, :],
                                    p2[:, tix, ts(cc, 128)],
                                    x_blk[:, tix, :],
                                    start=(jt0 + tix == 0),
                                    stop=(jt0 + tix == NJT - 1),
                                )
                pair_add(NPR - 1, p2_prev)

            def b4_drain(h, o_ps, sum_ps):
                srow = pool.tile([1, 512], F32, tag="row")
                nc.scalar.copy(srow[:], sum_ps[:])
                scol_ps = ps_qk.tile([128, 4], F32, tag="qk")
                for cc in range(4):
                    nc.tensor.transpose(
                        scol_ps[:, cc : cc + 1], srow[:, ts(cc, 128)], one_one[:]
                    )
                rcol = pool.tile([128, 4], F32, tag="rcol")
                nc.vector.reciprocal(rcol[:], scol_ps[:])
                for cc in range(4):
                    o_sb = wd.tile([128, 512], F32, tag="d")
                    if cc < 2:
                        nc.scalar.activation(
                            o_sb[:], o_ps[:, cc, :], AF.Copy,
                            bias=0.0, scale=rcol[:, cc : cc + 1],
                        )
                    else:
                        nc.vector.tensor_scalar_mul(
                            o_sb[:], o_ps[:, cc, :], rcol[:, cc : cc + 1]
                        )
                    nc.sync.dma_start(o[ts(h * 4 + cc, 128), :], o_sb[:])

            for h in range(NIH):
                st = big.tile([128, NJT, 512], F32, tag="big")
                mx = pool.tile([128, 512], F32, tag="mx")
                if h == 0:
                    # first block is resident (no stream slot), so R's second
                    # half can slot in behind it while its DMA completes
                    b1_block(h, st, mx, 0, ORDER[0])
                    r_half(1)
                    for pos in range(1, NBLK):
                        b1_block(h, st, mx, pos, ORDER[pos])
                else:
                    for pos in range(NBLK):
                        b1_block(h, st, mx, pos, ORDER[pos])
                b2_sb = finalize_max(mx)
                o_ps = ps_o.tile([128, 4, 512], F32, tag="o")
                sum_ps = ps_sum.tile([1, 512], F32, tag="sum")
                b3_exp_and_accum(st, b2_sb, o_ps, sum_ps)
                b4_drain(h, o_ps, sum_ps)

    nc.compile()
    return nc


def _get_nc():
    global _NC_CACHE
    if _NC_CACHE is None:
        _NC_CACHE = _build_nc()
    return _NC_CACHE


def kernel(rotation_params, entangle_params, inputs, _trace=False, _trace_kwargs=None):
    X = np.ascontiguousarray(inputs, dtype=np.float32)
    Wq = np.ascontiguousarray(rotation_params, dtype=np.float32)
    Wk = np.ascontiguousarray(entangle_params, dtype=np.float32)
    XT = np.ascontiguousarray(X.T)
    # blocked layouts: [p, blk, c, j] with 8KiB (f32) / 4KiB (f16) runs/partition
    XTB = XT.reshape(4, 128, NBLK, 512).transpose(1, 2, 0, 3)
    X16B = X.astype(np.float16).reshape(NBLK, 4, 128, 512).transpose(2, 0, 1, 3)
    wqt = np.ascontiguousarray(Wq.T)
    wkt = np.ascontiguousarray(Wk.T)

    in_maps = []
    for c in range(NCORES):
        # rotate the block axis so this core's own 2 blocks land at 0,1
        in_maps.append(
            {
                "xts": np.ascontiguousarray(np.roll(XTB, -2 * c, axis=1)),
                "xsb": np.ascontiguousarray(np.roll(X16B, -2 * c, axis=1)),
                "wqt": wqt,
                "wkt": wkt,
            }
        )

    nc = _get_nc()
    kw = {}
    if _trace:
        kw["trace"] = True
        kw.update(_trace_kwargs or {})
    br = run_bass_kernel_spmd(nc, in_maps, core_ids=list(range(NCORES)), **kw)
    out = np.concatenate([r["o"] for r in br.results], axis=0)
    if _trace:
        return out, br
    return out
